# revision 10
# baseline (speedup 1.0000x reference)
"""DeepAR LSTM kernel for 8 Trainium2 NeuronCores.

Strategy (data-parallel over batch, 256 -> 8 cores x 32):
  * Transposed state layout: hT/cT are [K=128 partitions, B_loc=32 free].
  * Per step: 8 small matmuls accumulate all 4 gates into one PSUM tile
    [128, 128] (free = 4 gates x 32 batch, gate order i,f,o,g so one
    sigmoid instruction covers i,f,o and one tanh covers g).
  * Bias (b_ih + b_hh) is folded into the x-side matmul by augmenting x
    with a constant-1 row (contraction 65 instead of 64).
  * The full hidden history HT [128, L*32] stays resident in SBUF; the
    mu/logsigma heads run as a post-pass: HT chunks are the *stationary*
    matmul operand against [W_mu.T | W_sig.T] [128, 64].
  * Head biases are added on the host (free).
"""

import os
import sys
from contextlib import ExitStack

import numpy as np

sys.path.insert(0, "/opt/trn_rl_repo")

import concourse.bass as bass
import concourse.tile as tile
from concourse import bacc, mybir
from concourse.bass_utils import run_bass_kernel_spmd

L, B, IN, K, OBS = 1024, 256, 64, 128, 32
NCORES = 8
BL = B // NCORES  # 32 batch rows per core
TC = 128          # x-chunk length in steps (DMA double-buffered)

_LSTEPS = int(os.environ.get("KERNEL_LSTEPS", L))  # smoke-test override

F32 = mybir.dt.float32
AF = mybir.ActivationFunctionType

_cache = {}
RUN_KW = {}         # test harness may inject trace=True/tmpdir
LAST_RESULT = None  # BassKernelResults of the most recent run


def build_nc(nsteps: int) -> bass.Bass:
    nc = bacc.Bacc(
        "TRN2", target_bir_lowering=False, debug=False, num_devices=NCORES
    )
    ntc = min(TC, nsteps)
    xt = nc.dram_tensor("xt", [IN + 1, nsteps * BL], F32, kind="ExternalInput")
    whh = nc.dram_tensor("whh_t", [K, 4 * K], F32, kind="ExternalInput")
    wih = nc.dram_tensor("wih_t", [IN + 1, 4 * K], F32, kind="ExternalInput")
    whd = nc.dram_tensor("wheads", [K, 2 * OBS], F32, kind="ExternalInput")
    heads = nc.dram_tensor(
        "heads", [nsteps * BL, 2 * OBS], F32, kind="ExternalOutput"
    )

    with ExitStack() as ctx:
        tc = ctx.enter_context(tile.TileContext(nc))
        singles = ctx.enter_context(tc.tile_pool(name="singles", bufs=1))
        xpool = ctx.enter_context(tc.tile_pool(name="xchunk", bufs=2))
        sgp = ctx.enter_context(tc.tile_pool(name="sg", bufs=3))
        cp = ctx.enter_context(tc.tile_pool(name="c", bufs=2))
        thp = ctx.enter_context(tc.tile_pool(name="th", bufs=2))
        tmpp = ctx.enter_context(tc.tile_pool(name="tmp", bufs=2))
        psp = ctx.enter_context(tc.tile_pool(name="ps", bufs=4, space="PSUM"))
        hpsp = ctx.enter_context(tc.tile_pool(name="hps", bufs=2, space="PSUM"))
        dpsp = ctx.enter_context(tc.tile_pool(name="dps", bufs=1, space="PSUM"))
        outp = ctx.enter_context(tc.tile_pool(name="outt", bufs=3))

        whh_sb = singles.tile([K, 4 * K], F32)
        nc.sync.dma_start(whh_sb[:], whh[:])
        wih_sb = singles.tile([IN + 1, 4 * K], F32)
        nc.sync.dma_start(wih_sb[:], wih[:])
        whd_sb = singles.tile([K, 2 * OBS], F32)
        nc.sync.dma_start(whd_sb[:], whd[:])
        HT = singles.tile([K, nsteps * BL], F32)

        # A matmul's LDWEIGHTS can carry only ONE sync wait; make PE
        # observe each DMA semaphore via a throwaway 1x1 matmul so real
        # matmuls never need a DMA wait on top of a compute wait.
        dummy_ps = dpsp.tile([1, 1], F32)
        absorb_state = {"first": True}

        def pe_absorb(tile_ap):
            nc.tensor.matmul(
                dummy_ps[:], tile_ap[0:1, 0:1], tile_ap[0:1, 0:1],
                start=absorb_state["first"], stop=False,
                skip_group_check=True,
            )
            absorb_state["first"] = False

        pe_absorb(whh_sb)
        pe_absorb(wih_sb)
        pe_absorb(whd_sb)

        cprev = None
        xt_tile = None
        for t in range(nsteps):
            if t % ntc == 0:
                xt_tile = xpool.tile([IN + 1, ntc * BL], F32)
                nc.sync.dma_start(
                    xt_tile[:], xt[:, t * BL : (t + ntc) * BL]
                )
                pe_absorb(xt_tile)
            xs = xt_tile[:, (t % ntc) * BL : (t % ntc + 1) * BL]
            ps = psp.tile([K, 4 * BL], F32)
            # Open the PSUM slot with a DVE corner-write: it absorbs the
            # ACT slot-release + PE drain waits (DVE instructions may
            # carry several waits, matmuls only one), so the matmuls
            # below need at most a single DVE wait.
            nc.vector.tensor_copy(ps[0:1, 0:1], wih_sb[0:1, 0:1])
            # x-side matmul first, h-side second (waits on DVE h write).
            for g in range(4):
                dst = ps[:, g * BL : (g + 1) * BL]
                if t == 0:
                    nc.tensor.matmul(
                        dst, wih_sb[:, g * K : (g + 1) * K], xs,
                        start=True, stop=True,
                    )
                else:
                    hprev = HT[:, (t - 1) * BL : t * BL]
                    nc.tensor.matmul(
                        dst, wih_sb[:, g * K : (g + 1) * K], xs,
                        start=True, stop=False,
                    )
                    nc.tensor.matmul(
                        dst, whh_sb[:, g * K : (g + 1) * K], hprev,
                        start=False, stop=True,
                    )
            sg = sgp.tile([K, 4 * BL], F32)
            nc.scalar.activation(sg[:, 0 : 3 * BL], ps[:, 0 : 3 * BL], AF.Sigmoid)
            nc.scalar.activation(
                sg[:, 3 * BL : 4 * BL], ps[:, 3 * BL : 4 * BL], AF.Tanh
            )
            cnew = cp.tile([K, BL], F32)
            if t == 0:
                nc.vector.tensor_mul(
                    cnew[:], sg[:, 0:BL], sg[:, 3 * BL : 4 * BL]
                )
            else:
                fc = tmpp.tile([K, BL], F32)
                nc.vector.tensor_mul(fc[:], sg[:, BL : 2 * BL], cprev[:])
                ig = tmpp.tile([K, BL], F32)
                nc.vector.tensor_mul(ig[:], sg[:, 0:BL], sg[:, 3 * BL : 4 * BL])
                nc.vector.tensor_add(cnew[:], fc[:], ig[:])
            th = thp.tile([K, BL], F32)
            nc.scalar.activation(th[:], cnew[:], AF.Tanh)
            nc.vector.tensor_mul(
                HT[:, t * BL : (t + 1) * BL], sg[:, 2 * BL : 3 * BL], th[:]
            )
            cprev = cnew

        # mu / logsigma heads: HT chunks as stationary operand.
        nch = nsteps * BL // K
        for m in range(nch):
            hps = hpsp.tile([K, 2 * OBS], F32)
            nc.tensor.matmul(
                hps[:], HT[:, m * K : (m + 1) * K], whd_sb[:],
                start=True, stop=True,
            )
            ot = outp.tile([K, 2 * OBS], F32)
            if m % 2 == 0:
                nc.vector.tensor_copy(ot[:], hps[:])
            else:
                nc.scalar.copy(ot[:], hps[:])
            nc.sync.dma_start(heads[m * K : (m + 1) * K, :], ot[:])
    nc.compile()
    return nc


def _prep_weights(W_ih, W_hh, b_ih, b_hh, W_mu, W_sig):
    # torch gate order in rows: i(0:K) f(K:2K) g(2K:3K) o(3K:4K)
    # reorder rows to (i, f, o, g) so sigmoid covers a contiguous block
    perm = np.r_[0:K, K : 2 * K, 3 * K : 4 * K, 2 * K : 3 * K]
    whh_t = np.ascontiguousarray(W_hh[perm].T, np.float32)          # [K, 4K]
    bias = (b_ih + b_hh)[perm].astype(np.float32)
    wih_t = np.concatenate(
        [W_ih[perm].T, bias[None, :]], axis=0
    ).astype(np.float32)                                            # [IN+1, 4K]
    wheads = np.concatenate([W_mu.T, W_sig.T], axis=1).astype(np.float32)
    return whh_t, wih_t, wheads


def kernel(external_input_seq, W_ih, W_hh, b_ih, b_hh, W_mu, b_mu, W_sig, b_sig):
    nsteps = _LSTEPS
    x = np.asarray(external_input_seq, np.float32)[:nsteps]
    W_ih = np.asarray(W_ih, np.float32)
    W_hh = np.asarray(W_hh, np.float32)
    b_ih = np.asarray(b_ih, np.float32)
    b_hh = np.asarray(b_hh, np.float32)
    W_mu = np.asarray(W_mu, np.float32)
    b_mu = np.asarray(b_mu, np.float32)
    W_sig = np.asarray(W_sig, np.float32)
    b_sig = np.asarray(b_sig, np.float32)

    whh_t, wih_t, wheads = _prep_weights(W_ih, W_hh, b_ih, b_hh, W_mu, W_sig)

    if nsteps not in _cache:
        _cache[nsteps] = build_nc(nsteps)
    nc = _cache[nsteps]

    in_maps = []
    for c in range(NCORES):
        xc = x[:, c * BL : (c + 1) * BL, :]              # [nsteps, BL, IN]
        xt = np.empty((IN + 1, nsteps * BL), np.float32)
        xt[:IN] = xc.transpose(2, 0, 1).reshape(IN, nsteps * BL)
        xt[IN] = 1.0
        in_maps.append(
            {"xt": xt, "whh_t": whh_t, "wih_t": wih_t, "wheads": wheads}
        )

    res = run_bass_kernel_spmd(
        nc, in_maps, core_ids=list(range(NCORES)), **RUN_KW
    )
    global LAST_RESULT
    LAST_RESULT = res

    mu = np.empty((nsteps, B, OBS), np.float32)
    sig = np.empty((nsteps, B, OBS), np.float32)
    for c in range(NCORES):
        h = res.results[c]["heads"].reshape(nsteps, BL, 2 * OBS)
        mu[:, c * BL : (c + 1) * BL, :] = h[:, :, :OBS]
        sig[:, c * BL : (c + 1) * BL, :] = h[:, :, OBS:]
    mu += b_mu
    sig += b_sig
    return mu, sig
